# revision 7
# baseline (speedup 1.0000x reference)
"""Trainium2 Bass kernel for a binarized 4-layer MLP (eval mode).

Reference computation (per row of x [B=16384, 784]):
  h1 = x @ sign(w1).T + b1;  s1 = sign(bn1(h1))        (clip doesn't change sign)
  h2 = s1 @ sign(w2).T + b2; s2 = sign(bn2(h2))
  h3 = s2 @ sign(w3).T + b3; y3 = clip(bn3(h3), -1, 1)
  z  = y3 @ w4.T + b4;       out = log_softmax(z)

Sharding: pure data-parallel over the batch across 8 NeuronCores
(weights replicated, no collectives).

Numerics:
  - L1 splits x on the HOST into a fp16 main stream plus a scaled fp8
    residual rb = fp8e4((x - fp16(x)) * 2^9); the residual's stationary
    operand is sign(w1) * 2^-9 (exact in fp8e4), accumulated into the
    same fp32 PSUM.  Rows 0:768 of the residual ride 3 DoubleRow
    passes; rows 768:784 ride IN the fp16 stream as extra fp16 rows
    (fp16 of the fp32 remainder -- err 2^-22|x| there), so the 7th
    fp16 pass is 128 rows: x[672:784] ++ r[768:784].  Total L1 error
    <= 2^-15|x| -> 7.6e-3 end-to-end (tolerance 2e-2).
  - L2/L3: operands are +-1/0 in fp8e4 -> DoubleRow matmuls are
    bit-exact integer sums in fp32 PSUM.
  - BN + bias folding: bn(h + b) = A*h + C applied per-partition by the
    Sign/Identity activations (fp32 internally).

Schedule notes (from NTFF trace analysis):
  - All fp8 matmuls use DoubleRowSwInterleave at N=512: the plain-DR
    +0.1ns/col penalty at 512 wide is gone with SW-interleaved weights
    (HW-probed 215.8ns at N=512, same 0.42ns/col as fp16), and the
    ~135ns DR LDWEIGHTS hides under 216ns matmuls -- the baseline's
    256-wide residual paid ~25us of un-hidden LDWEIGHTS bubbles.
  - Weights for SW-interleave are pre-interleaved on the host as exact
    SBUF images (per column pair j: A[:, ::-1]/B[:, ::-1] interleaved),
    so their DMA is a single contiguous transfer.
  - every dma_start costs ~620ns of serialized Sync-engine time, so
    inputs/weights load as FEW large transfers; only the group-0
    block-0 critical path keeps small fine-grained transfers.
  - a dummy-matmul burst on a zero tile warms the PE HAM clock gate
    (1.2 -> 2.4 GHz) while the startup DMAs are still in flight.
  - h1/h2 activations are split into two tiles each so the next
    layer's first matmul depends only on the first half.
  - the log-softmax epilogue for group g-1 hides under group g; in the
    last group it is interleaved per batch-tile behind L4.
"""

import sys

if "/opt/trn_rl_repo" not in sys.path:
    sys.path.insert(0, "/opt/trn_rl_repo")

import numpy as np

D_IN, H1, H2, H3, NCLS = 784, 3072, 1536, 768, 10
B, NCORES = 16384, 8
BC = B // NCORES          # batch rows per core
W = 512                   # batch columns per group
NG = BC // W              # groups per core
KP = 112                  # L1 fp16 k-tile partition size for passes 0-5
K1A = 6                   # fp16 112-row passes (rows 0:672)
QR = 3                    # residual DoubleRow passes (rows 0:768)
M1, M2, M3 = H1 // 128, H2 // 128, H3 // 128   # 24, 12, 6
K2P, K3P = H1 // 256, H2 // 256                # DR pair-pass iters: 12, 6
K4T = H3 // 128                                # 6
NCST = 2 * M1 + 2 * M2 + 2 * M3 + NCLS         # packed per-tile consts: 94
BN_EPS = 1e-5
RSH = 9                   # residual scale: rb = fp8(r * 2^RSH), w = +-2^-RSH
NWARM = 3
NWARM2 = 20

_cached = {}


def _build(bc):
    import concourse.bacc as bacc
    import concourse.mybir as mybir
    import concourse.tile as tile

    dt = mybir.dt
    AF = mybir.ActivationFunctionType
    PM = mybir.MatmulPerfMode
    ALU = mybir.AluOpType
    DRSW = PM.DoubleRowSwInterleave

    ng = bc // W
    nc = bacc.Bacc("TRN2", target_bir_lowering=False, debug=False,
                   num_devices=NCORES)

    # xa6 = fp16(x).T rows 0:672 per core, host image [112, 6*bc]
    xa6 = nc.declare_dram_parameter("xa6", [KP, K1A * bc], dt.float16,
                                    isOutput=False)
    # xa7 = fp16 pass-7 rows: x.T[672:784] ++ r.T[768:784], [128, bc]
    xa7 = nc.declare_dram_parameter("xa7", [128, bc], dt.float16,
                                    isOutput=False)
    # xr = fp8((x - fp16(x)).T[0:768] * 2^RSH) DR-paired, image [128, 6*bc]
    xr = nc.declare_dram_parameter("xr", [128, 2 * QR * bc], dt.float8e4,
                                   isOutput=False)
    # w1a = sign(w1).T rows 0:672 fp16, image [112, 6*3072]
    w1a = nc.declare_dram_parameter("w1a", [KP, K1A * H1], dt.float16,
                                    isOutput=False)
    # w1b = sign(w1).T rows 672:784 ++ 768:784, [128, 3072]
    w1b = nc.declare_dram_parameter("w1b", [128, H1], dt.float16,
                                    isOutput=False)
    # w1sw = residual stationary, SW-interleaved image [128, 24*3*256] fp8
    # (mt-major so per-4-tile-block transfers are contiguous)
    w1sw = nc.declare_dram_parameter("w1sw", [128, M1 * QR * 256],
                                     dt.float8e4, isOutput=False)
    w2sw = nc.declare_dram_parameter("w2sw", [128, K2P * M2 * 256],
                                     dt.float8e4, isOutput=False)
    w3sw = nc.declare_dram_parameter("w3sw", [128, K3P * M3 * 256],
                                     dt.float8e4, isOutput=False)
    w4t = nc.declare_dram_parameter("w4t", [H3, NCLS], dt.bfloat16,
                                    isOutput=False)
    cst = nc.declare_dram_parameter("cst", [128, NCST], dt.float32,
                                    isOutput=False)
    wrm = nc.declare_dram_parameter("wrm", [128, W], dt.float16,
                                    isOutput=False)
    out = nc.declare_dram_parameter("out", [bc, NCLS], dt.float32,
                                    isOutput=True)

    with tile.TileContext(nc) as tc, \
            tc.tile_pool(name="wts", bufs=1) as wp, \
            tc.tile_pool(name="act", bufs=1) as ap_, \
            tc.tile_pool(name="eps", bufs=2) as ep, \
            tc.tile_pool(name="ps", bufs=4, space="PSUM") as ps, \
            tc.tile_pool(name="ps4", bufs=2, space="PSUM") as ps4, \
            tc.tile_pool(name="psw", bufs=1, space="PSUM") as pw:

        # ---- HAM warm-up while the startup DMAs stream.
        wrmb = wp.tile([128, W], dt.float16, tag="wrm")
        nc.sync.dma_start(wrmb[:], wrm[:])
        pwt = pw.tile([128, W], dt.float32, tag="pw")
        for wi in range(NWARM):
            nc.tensor.matmul(pwt[:], wrmb[:, 0:128], wrmb[:],
                             start=(wi == 0), stop=(wi == NWARM - 1))

        # ---- startup-critical transfers first: group-0 x and the first
        # 512-col block of w1, fine-grained; then per-block w1; then bulk.
        xa6v = xa6.ap().rearrange("p (k w) -> p k w", k=K1A)
        w1av = w1a.ap().rearrange("p (k n) -> p k n", k=K1A)
        xrv = xr.ap().rearrange("p (j w) -> p j w", j=2 * QR)
        w1swv = w1sw.ap().rearrange("p (m q n) -> p m q n", m=M1, q=QR)
        w2swv = w2sw.ap().rearrange("p (q m n) -> p q m n", q=K2P, m=M2)
        w3swv = w3sw.ap().rearrange("p (q m n) -> p q m n", q=K3P, m=M3)

        x00 = wp.tile([KP, W], dt.float16, tag="x00")
        nc.sync.dma_start(x00[:], xa6v[:, 0, 0:W])
        w1k0a = wp.tile([KP, W], dt.float16, tag="w1k0a")
        nc.sync.dma_start(w1k0a[:], w1av[:, 0, 0:W])
        x0r = wp.tile([KP, K1A - 1, W], dt.float16, tag="x0r")
        nc.sync.dma_start(x0r[:], xa6v[:, 1:K1A, 0:W])
        w1ka = wp.tile([KP, K1A - 1, W], dt.float16, tag="w1ka")
        nc.sync.dma_start(w1ka[:], w1av[:, 1:K1A, 0:W])
        x70 = wp.tile([128, W], dt.float16, tag="x70")
        nc.sync.dma_start(x70[:], xa7[:, 0:W])
        w1b0 = wp.tile([128, W], dt.float16, tag="w1b0")
        nc.sync.dma_start(w1b0[:], w1b[:, 0:W])
        cstb = wp.tile([128, NCST], dt.float32, tag="cst")
        nc.sync.dma_start(cstb[:], cst[:])
        xr0 = wp.tile([128, 2 * QR, W], dt.float8e4, tag="xr0")
        nc.sync.dma_start(xr0[:], xrv[:, :, 0:W])
        # residual stationary per 4-tile block (block 0 first)
        w1swcb = []
        t = wp.tile([128, 4, QR, 256], dt.float8e4, tag="w1sw0",
                    name="w1sw0")
        nc.sync.dma_start(t[:], w1swv[:, 0:4, :, :])
        w1swcb.append(t)
        # remaining w1 fp16 + residual stationary per 512-col block,
        # paced to group-0 use
        w1cb = [None]
        w1bb = [w1b0]
        for b in range(1, M1 // 4):
            t = wp.tile([KP, K1A, W], dt.float16, tag=f"w1c{b}",
                        name=f"w1c{b}")
            nc.sync.dma_start(t[:], w1av[:, :, b * W:(b + 1) * W])
            w1cb.append(t)
            t = wp.tile([128, W], dt.float16, tag=f"w1b{b}",
                        name=f"w1b{b}")
            nc.sync.dma_start(t[:], w1b[:, b * W:(b + 1) * W])
            w1bb.append(t)
            t = wp.tile([128, 4, QR, 256], dt.float8e4, tag=f"w1sw{b}",
                        name=f"w1sw{b}")
            nc.sync.dma_start(t[:], w1swv[:, 4 * b:4 * b + 4, :, :])
            w1swcb.append(t)

        def w1v(k, mt):
            b, i = mt // 4, mt % 4
            cs = slice(i * 128, (i + 1) * 128)
            if k == K1A:
                return w1bb[b][:, cs]
            if b == 0:
                return w1k0a[:, cs] if k == 0 else w1ka[:, k - 1, cs]
            return w1cb[b][:, k, cs]

        # x for groups 1..3, one transfer each stream
        xab = wp.tile([KP, K1A, (ng - 1) * W], dt.float16, tag="xab")
        nc.sync.dma_start(xab[:], xa6v[:, :, W:bc])
        xa7b = wp.tile([128, (ng - 1) * W], dt.float16, tag="xa7b")
        nc.sync.dma_start(xa7b[:], xa7[:, W:bc])
        xr123 = wp.tile([128, 2 * QR, (ng - 1) * W], dt.float8e4,
                        tag="xr123")
        nc.sync.dma_start(xr123[:], xrv[:, :, W:bc])

        w2swb = wp.tile([128, K2P, M2, 256], dt.float8e4, tag="w2sw")
        nc.sync.dma_start(w2swb[:], w2swv[:])
        w3swb = wp.tile([128, K3P, M3, 256], dt.float8e4, tag="w3sw")
        nc.sync.dma_start(w3swb[:], w3swv[:])
        w4sb = wp.tile([128, K4T, NCLS], dt.bfloat16, tag="w4")
        nc.sync.dma_start(w4sb[:],
                          w4t.ap().rearrange("(kt p) n -> p kt n", p=128))

        # const views into the packed per-output-tile scale/bias table
        def a1v(mt): return cstb[:, mt:mt + 1]
        def c1v(mt): return cstb[:, M1 + mt:M1 + mt + 1]
        def a2v(mt): return cstb[:, 2 * M1 + mt:2 * M1 + mt + 1]
        def c2v(mt): return cstb[:, 2 * M1 + M2 + mt:2 * M1 + M2 + mt + 1]
        def a3v(mt):
            o = 2 * M1 + 2 * M2
            return cstb[:, o + mt:o + mt + 1]
        def c3v(mt):
            o = 2 * M1 + 2 * M2 + M3
            return cstb[:, o + mt:o + mt + 1]
        b4v = cstb[:, NCST - NCLS:NCST]

        zout = wp.tile([128, ng * 4, NCLS], dt.float32, tag="zout")
        ssum = wp.tile([128, ng * 4], dt.float32, tag="ssum")
        lsum = wp.tile([128, ng * 4], dt.float32, tag="lsum")

        def emit_epilogue(lo, hi, dma=True):
            # log_softmax over the free dim; |z| is small so no max-shift
            for r in range(lo, hi):
                e = ep.tile([128, NCLS], dt.float32, tag="e")
                nc.scalar.activation(e[:], zout[:, r, :], AF.Exp,
                                     accum_out=ssum[:, r:r + 1])
            nc.scalar.activation(lsum[:, lo:hi], ssum[:, lo:hi], AF.Ln)
            for r in range(lo, hi):
                nc.vector.tensor_scalar(zout[:, r, :], zout[:, r, :],
                                        lsum[:, r:r + 1], None,
                                        op0=ALU.subtract)
            if dma:
                nc.sync.dma_start(
                    out.ap()[lo * 128:hi * 128, :].rearrange(
                        "(g p) n -> p g n", p=128),
                    zout[:, lo:hi, :])

        def xav(k, g):
            if g == 0:
                if k == K1A:
                    return x70[:]
                return x00[:] if k == 0 else x0r[:, k - 1, :]
            if k == K1A:
                return xa7b[:, (g - 1) * W:g * W]
            return xab[:, k, (g - 1) * W:g * W]

        for g in range(ng):
            if g > 0:
                # epilogue for the previous group hides under this
                # group's L1 matmuls
                emit_epilogue(4 * (g - 1), 4 * g)

            xrg = xr0 if g == 0 else xr123
            off = 0 if g == 0 else (g - 1) * W

            # ---- L1: [784 -> 3072], 6x112-row + 1x128-row fp16 passes +
            # 3 DRSW fp8 residual passes, all N=512
            h1t = [ap_.tile([128, K2P, W], dt.float8e4, tag="h1a",
                            name="h1a"),
                   ap_.tile([128, K2P, W], dt.float8e4, tag="h1b",
                            name="h1b")]
            for mt in range(M1):
                pt = ps.tile([128, W], dt.float32, tag="ps")
                for k in range(K1A + 1):
                    nc.tensor.matmul(pt[:], w1v(k, mt), xav(k, g),
                                     start=(k == 0), stop=False)
                    if g == 0 and mt == 0 and k == 0:
                        # second warm burst: keeps the PE busy through the
                        # data-wait hole while the rest of the group-0
                        # transfers land
                        for wi in range(NWARM2):
                            nc.tensor.matmul(pwt[:], wrmb[:, 0:128],
                                             wrmb[:], start=(wi == 0),
                                             stop=(wi == NWARM2 - 1))
                for q in range(QR):
                    nc.tensor.matmul(pt[:], w1swcb[mt // 4][:, mt % 4, q, :],
                                     xrg[:, 2 * q:2 * q + 2,
                                         off:off + W],
                                     start=False, stop=(q == QR - 1),
                                     perf_mode=DRSW)
                nc.scalar.activation(h1t[mt // 12][:, mt % 12, :], pt[:],
                                     AF.Sign, bias=c1v(mt), scale=a1v(mt))

            # ---- L2: [3072 -> 1536], fp8 DRSW, N=512
            h2t = [ap_.tile([128, K3P, W], dt.float8e4, tag="h2a",
                            name="h2a"),
                   ap_.tile([128, K3P, W], dt.float8e4, tag="h2b",
                            name="h2b")]
            for mt in range(M2):
                pt = ps.tile([128, W], dt.float32, tag="ps")
                for kp in range(K2P):
                    nc.tensor.matmul(
                        pt[:], w2swb[:, kp, mt, :],
                        h1t[kp // 6][:, 2 * (kp % 6):2 * (kp % 6) + 2, :],
                        start=(kp == 0), stop=(kp == K2P - 1),
                        perf_mode=DRSW)
                nc.scalar.activation(h2t[mt // 6][:, mt % 6, :], pt[:],
                                     AF.Sign, bias=c2v(mt), scale=a2v(mt))

            # ---- L3: [1536 -> 768], fp8 DRSW, N=512; scale/bias on the
            # Scalar engine (Identity), clip on DVE; bf16 output keeps
            # L4 single-pass
            h3t = [ap_.tile([128, K4T // 2, W], dt.bfloat16,
                            tag="h3a", name="h3a"),
                   ap_.tile([128, K4T // 2, W], dt.bfloat16,
                            tag="h3b", name="h3b")]
            for mt in range(M3):
                pt = ps.tile([128, W], dt.float32, tag="ps")
                for kp in range(K3P):
                    nc.tensor.matmul(
                        pt[:], w3swb[:, kp, mt, :],
                        h2t[kp // 3][:, 2 * (kp % 3):2 * (kp % 3) + 2, :],
                        start=(kp == 0), stop=(kp == K3P - 1),
                        perf_mode=DRSW)
                h3v = h3t[mt // 3][:, mt % 3, :]
                nc.scalar.activation(h3v, pt[:], AF.Identity,
                                     bias=c3v(mt), scale=a3v(mt))
                nc.vector.tensor_scalar(h3v, h3v, 1.0, -1.0,
                                        op0=ALU.min, op1=ALU.max)

            # ---- L4: logits z = y3 @ w4.T + b4, [batch-tile, 10]
            for bt in range(4):
                r = 4 * g + bt
                p4 = ps4.tile([128, NCLS], dt.float32, tag="p4")
                for kt in range(K4T):
                    nc.tensor.matmul(p4[:],
                                     h3t[kt // 3][:, kt % 3,
                                         bt * 128:(bt + 1) * 128],
                                     w4sb[:, kt, :],
                                     start=(kt == 0),
                                     stop=(kt == K4T - 1))
                nc.vector.tensor_add(zout[:, r, :], p4[:], b4v)
                if g == ng - 1:
                    # last group: per-tile epilogue rides behind the
                    # next batch-tile's L4 matmuls; one batched out
                    # DMA at the end
                    emit_epilogue(r, r + 1, dma=False)

        nc.sync.dma_start(
            out.ap()[(ng * 4 - 4) * 128:ng * 4 * 128, :].rearrange(
                "(g p) n -> p g n", p=128),
            zout[:, ng * 4 - 4:ng * 4, :])

    nc.finalize()
    return nc


def _interleave(A, Bm):
    """SW-interleave layout for DoubleRowSwInterleave stationary:
    L[:, 0::2] = A[:, ::-1]; L[:, 1::2] = B[:, ::-1]."""
    p, m = A.shape
    L = np.empty((p, 2 * m), A.dtype)
    L[:, 0::2] = A[:, ::-1]
    L[:, 1::2] = Bm[:, ::-1]
    return L


def _sw_image(wt, kpairs, mtiles):
    """Build the [128, kpairs*mtiles*256] SW-interleaved image from
    wt [K, M] (K = 256*kpairs contraction rows, M = 128*mtiles outs)."""
    K, M = wt.shape
    assert K == 256 * kpairs and M == 128 * mtiles
    img = np.empty((128, kpairs * mtiles * 256), np.float32)
    v = img.reshape(128, kpairs, mtiles, 256)
    for q in range(kpairs):
        A = wt[256 * q:256 * q + 128, :]
        Bm = wt[256 * q + 128:256 * q + 256, :]
        for mt in range(mtiles):
            v[:, q, mt, :] = _interleave(A[:, mt * 128:(mt + 1) * 128],
                                         Bm[:, mt * 128:(mt + 1) * 128])
    return img


def _prep(x, w1, b1, w2, b2, w3, b3, w4, b4,
          g1, be1, m1, v1, g2, be2, m2, v2, g3, be3, m3, v3):
    """Host-side prep: transposes, binarized weight casts, BN folds,
    the fp16 + scaled-fp8 split of x, and SW-interleaved fp8 images."""
    import concourse.mybir as mybir
    f8 = mybir.dt.np(mybir.dt.float8e4)
    bf16 = mybir.dt.np(mybir.dt.bfloat16)

    def fold(g, be, m, v, b):
        a = (g / np.sqrt(v + np.float32(BN_EPS))).astype(np.float32)
        c = (a * (b - m) + be).astype(np.float32)
        return a, c

    a1, c1 = fold(g1, be1, m1, v1, b1)
    a2, c2 = fold(g2, be2, m2, v2, b2)
    a3, c3 = fold(g3, be3, m3, v3, b3)

    def cols(v, mtiles):
        return v.reshape(mtiles, 128).T

    cstm = np.zeros((128, NCST), np.float32)
    o = 0
    for v, m in ((a1, M1), (c1, M1), (a2, M2), (c2, M2), (a3, M3), (c3, M3)):
        cstm[:, o:o + m] = cols(v, m)
        o += m
    cstm[:, o:o + NCLS] = b4.astype(np.float32)[None, :]

    s1t = np.sign(w1).T.astype(np.float32)          # [784, 3072]
    # w1a image: rows 0:672 as [112, 6*3072] (k-major per partition)
    w1a = np.ascontiguousarray(
        s1t[0:672].reshape(K1A, KP, H1).transpose(1, 0, 2)
        .reshape(KP, K1A * H1)).astype(np.float16)
    # w1b: rows 672:784 ++ rows 768:784 (for the fp16 residual tail)
    w1b = np.ascontiguousarray(
        np.concatenate([s1t[672:784], s1t[768:784]], axis=0)
    ).astype(np.float16)
    # w1sw: residual stationary rows 0:768, scaled 2^-RSH; mt-major
    w1sw = np.ascontiguousarray(
        _sw_image(s1t[0:768] * np.float32(2.0 ** -RSH), QR, M1)
        .reshape(128, QR, M1, 256).transpose(0, 2, 1, 3)
        .reshape(128, M1 * QR * 256))

    pre = dict(
        w1a=w1a, w1b=w1b,
        w1sw=w1sw.astype(f8),
        w2sw=_sw_image(np.sign(w2).T.astype(np.float32), K2P, M2).astype(f8),
        w3sw=_sw_image(np.sign(w3).T.astype(np.float32), K3P, M3).astype(f8),
        w4t=np.ascontiguousarray(w4.T).astype(bf16),
        cst=cstm,
        wrm=np.zeros((128, W), np.float16),
    )

    x = x.astype(np.float32)
    xa16 = x.astype(np.float16)
    r = x - xa16.astype(np.float32)
    # xa6 image: x.T rows 0:672 as [112, 6*bc-per-core] built per core later
    xa6t = xa16.T[0:672]                            # [672, B]
    # pass-7 rows: x.T[672:784] ++ fp16(r.T[768:784])
    xa7t = np.concatenate([xa16.T[672:784],
                           r.T[768:784].astype(np.float16)], axis=0)
    rb = (r.T[0:768] * np.float32(2.0 ** RSH)).astype(f8)   # [768, B]
    return pre, xa6t, xa7t, rb


def run(inputs, **spmd_kwargs):
    from concourse.bass_utils import run_bass_kernel_spmd

    if "nc" not in _cached:
        _cached["nc"] = _build(BC)
    nc = _cached["nc"]

    inputs = {k: np.asarray(v) for k, v in inputs.items()}
    pre, xa6t, xa7t, rb = _prep(**inputs)

    in_maps = []
    for core in range(NCORES):
        m = dict(pre)
        cs = slice(core * BC, (core + 1) * BC)
        m["xa6"] = np.ascontiguousarray(
            xa6t[:, cs].reshape(K1A, KP, BC).transpose(1, 0, 2)
            .reshape(KP, K1A * BC))
        m["xa7"] = np.ascontiguousarray(xa7t[:, cs])
        # xr image: [128, 6*bc], j = 2q+e -> rows 256q+128e+p
        m["xr"] = np.ascontiguousarray(
            rb[:, cs].reshape(2 * QR, 128, BC).transpose(1, 0, 2)
            .reshape(128, 2 * QR * BC))
        in_maps.append(m)

    res = run_bass_kernel_spmd(nc, in_maps, list(range(NCORES)), **spmd_kwargs)
    outs = [res.results[i]["out"] for i in range(NCORES)]
    return res, np.concatenate(outs, axis=0).astype(np.float32)


def kernel(**inputs):
    return run(inputs)[1]


# revision 10
# speedup vs baseline: 1.2195x; 1.2195x over previous
"""Trainium2 Bass kernel for a binarized 4-layer MLP (eval mode).

Reference computation (per row of x [B=16384, 784]):
  h1 = x @ sign(w1).T + b1;  s1 = sign(bn1(h1))        (clip doesn't change sign)
  h2 = s1 @ sign(w2).T + b2; s2 = sign(bn2(h2))
  h3 = s2 @ sign(w3).T + b3; y3 = clip(bn3(h3), -1, 1)
  z  = y3 @ w4.T + b4;       out = log_softmax(z)

Sharding: pure data-parallel over the batch across 8 NeuronCores
(weights replicated, no collectives).

Numerics:
  - L1 splits x on the HOST into a fp16 main stream plus a scaled fp8
    residual rb = fp8e4((x - fp16(x)) * 2^9); the residual's stationary
    operand is sign(w1) * 2^-9 (exact in fp8e4), accumulated into the
    same fp32 PSUM.  Rows 0:768 of the residual ride 3 DoubleRow
    passes; rows 768:784 ride IN the fp16 stream as extra fp16 rows
    (fp16 of the fp32 remainder -- err 2^-22|x| there), so the 7th
    fp16 pass is 128 rows: x[672:784] ++ r[768:784].  Total L1 error
    <= 2^-15|x| -> 7.6e-3 end-to-end (tolerance 2e-2).
  - L2/L3: operands are +-1/0 in fp8e4 -> DoubleRow matmuls are
    bit-exact integer sums in fp32 PSUM.
  - BN + bias folding: bn(h + b) = A*h + C applied per-partition by the
    Sign/Identity activations (fp32 internally).

Schedule notes (from NTFF trace analysis):
  - All fp8 matmuls use DoubleRowSwInterleave at N=512: the plain-DR
    +0.1ns/col penalty at 512 wide is gone with SW-interleaved weights
    (HW-probed 215.8ns at N=512, same 0.42ns/col as fp16), and the
    ~135ns DR LDWEIGHTS hides under 216ns matmuls -- the baseline's
    256-wide residual paid ~25us of un-hidden LDWEIGHTS bubbles.
  - Weights for SW-interleave are pre-interleaved on the host as exact
    SBUF images (per column pair j: A[:, ::-1]/B[:, ::-1] interleaved),
    so their DMA is a single contiguous transfer.
  - every dma_start costs ~620ns of serialized Sync-engine time, so
    inputs/weights load as FEW large transfers; only the group-0
    block-0 critical path keeps small fine-grained transfers.
  - a dummy-matmul burst on a zero tile warms the PE HAM clock gate
    (1.2 -> 2.4 GHz) while the startup DMAs are still in flight.
  - h1/h2 activations are split into two tiles each so the next
    layer's first matmul depends only on the first half.
  - the log-softmax epilogue for group g-1 hides under group g; in the
    last group it is interleaved per batch-tile behind L4.
"""

import sys

if "/opt/trn_rl_repo" not in sys.path:
    sys.path.insert(0, "/opt/trn_rl_repo")

import numpy as np

D_IN, H1, H2, H3, NCLS = 784, 3072, 1536, 768, 10
B, NCORES = 16384, 8
BC = B // NCORES          # batch rows per core
W = 512                   # batch columns per group
NG = BC // W              # groups per core
KP = 112                  # L1 fp16 k-tile partition size for passes 0-5
K1A = 6                   # fp16 112-row passes (rows 0:672)
QR = 3                    # residual DoubleRow passes (rows 0:768)
M1, M2, M3 = H1 // 128, H2 // 128, H3 // 128   # 24, 12, 6
K2P, K3P = H1 // 256, H2 // 256                # DR pair-pass iters: 12, 6
K4T = H3 // 128                                # 6
NCST = 2 * M1 + 2 * M2 + 2 * M3 + NCLS         # packed per-tile consts: 94
BN_EPS = 1e-5
RSH = 9                   # residual scale: rb = fp8(r * 2^RSH), w = +-2^-RSH
NWARM = 3
NWARM2 = 10

_cached = {}


def _build(bc):
    import concourse.bacc as bacc
    import concourse.mybir as mybir
    import concourse.tile as tile

    dt = mybir.dt
    AF = mybir.ActivationFunctionType
    PM = mybir.MatmulPerfMode
    ALU = mybir.AluOpType
    DRSW = PM.DoubleRowSwInterleave

    ng = bc // W
    nc = bacc.Bacc("TRN2", target_bir_lowering=False, debug=False,
                   num_devices=NCORES)

    # xa6 = fp16(x).T rows 0:672 per core, host image [112, 6*bc]
    xa6 = nc.declare_dram_parameter("xa6", [KP, K1A * bc], dt.float16,
                                    isOutput=False)
    # xa7 = fp16 pass-7 rows: x.T[672:784] ++ r.T[768:784], [128, bc]
    xa7 = nc.declare_dram_parameter("xa7", [128, bc], dt.float16,
                                    isOutput=False)
    # xr = fp8((x - fp16(x)).T[0:768] * 2^RSH) DR-paired, image [128, 6*bc]
    xr = nc.declare_dram_parameter("xr", [128, 2 * QR * bc], dt.float8e4,
                                   isOutput=False)
    # w1a = sign(w1).T rows 0:672 fp16, image [112, 6*3072]
    w1a = nc.declare_dram_parameter("w1a", [KP, K1A * H1], dt.float16,
                                    isOutput=False)
    # w1b = sign(w1).T rows 672:784 ++ 768:784, [128, 3072]
    w1b = nc.declare_dram_parameter("w1b", [128, H1], dt.float16,
                                    isOutput=False)
    # w1sw = residual stationary, SW-interleaved image [128, 24*3*256] fp8
    # (mt-major so per-4-tile-block transfers are contiguous)
    w1sw = nc.declare_dram_parameter("w1sw", [128, M1 * QR * 256],
                                     dt.float8e4, isOutput=False)
    w2sw = nc.declare_dram_parameter("w2sw", [128, K2P * M2 * 256],
                                     dt.float8e4, isOutput=False)
    w3sw = nc.declare_dram_parameter("w3sw", [128, K3P * M3 * 256],
                                     dt.float8e4, isOutput=False)
    w4t = nc.declare_dram_parameter("w4t", [H3, NCLS], dt.bfloat16,
                                    isOutput=False)
    cst = nc.declare_dram_parameter("cst", [128, NCST], dt.float32,
                                    isOutput=False)
    wrm = nc.declare_dram_parameter("wrm", [128, W], dt.float16,
                                    isOutput=False)
    out = nc.declare_dram_parameter("out", [bc, NCLS], dt.float32,
                                    isOutput=True)

    with tile.TileContext(nc) as tc, \
            tc.tile_pool(name="wts", bufs=1) as wp, \
            tc.tile_pool(name="act", bufs=1) as ap_, \
            tc.tile_pool(name="eps", bufs=2) as ep, \
            tc.tile_pool(name="ps", bufs=4, space="PSUM") as ps, \
            tc.tile_pool(name="ps4", bufs=2, space="PSUM") as ps4, \
            tc.tile_pool(name="psw", bufs=1, space="PSUM") as pw:

        # ---- HAM warm-up while the startup DMAs stream.
        wrmb = wp.tile([128, W], dt.float16, tag="wrm")
        nc.sync.dma_start(wrmb[:], wrm[:])
        pwt = pw.tile([128, W], dt.float32, tag="pw")
        for wi in range(NWARM):
            nc.tensor.matmul(pwt[:], wrmb[:, 0:128], wrmb[:],
                             start=(wi == 0), stop=(wi == NWARM - 1))

        # ---- startup-critical transfers first: group-0 x and the first
        # 512-col block of w1, fine-grained; then per-block w1; then bulk.
        xa6v = xa6.ap().rearrange("p (k w) -> p k w", k=K1A)
        w1av = w1a.ap().rearrange("p (k n) -> p k n", k=K1A)
        xrv = xr.ap().rearrange("p (j w) -> p j w", j=2 * QR)
        w1swv = w1sw.ap().rearrange("p (m q n) -> p m q n", m=M1, q=QR)
        w2swv = w2sw.ap().rearrange("p (q m n) -> p q m n", q=K2P, m=M2)
        w3swv = w3sw.ap().rearrange("p (q m n) -> p q m n", q=K3P, m=M3)

        x00 = wp.tile([KP, W], dt.float16, tag="x00")
        nc.sync.dma_start(x00[:], xa6v[:, 0, 0:W])
        w1k0a = wp.tile([KP, W], dt.float16, tag="w1k0a")
        nc.sync.dma_start(w1k0a[:], w1av[:, 0, 0:W])
        x0r = wp.tile([KP, K1A - 1, W], dt.float16, tag="x0r")
        nc.sync.dma_start(x0r[:], xa6v[:, 1:K1A, 0:W])
        w1ka = wp.tile([KP, K1A - 1, W], dt.float16, tag="w1ka")
        nc.sync.dma_start(w1ka[:], w1av[:, 1:K1A, 0:W])
        x70 = wp.tile([128, W], dt.float16, tag="x70")
        nc.sync.dma_start(x70[:], xa7[:, 0:W])
        w1b0 = wp.tile([128, W], dt.float16, tag="w1b0")
        nc.sync.dma_start(w1b0[:], w1b[:, 0:W])
        cstb = wp.tile([128, NCST], dt.float32, tag="cst")
        nc.sync.dma_start(cstb[:], cst[:])
        xr0 = wp.tile([128, 2 * QR, W], dt.float8e4, tag="xr0")
        nc.sync.dma_start(xr0[:], xrv[:, :, 0:W])
        # residual stationary per 4-tile block (block 0 first)
        w1swcb = []
        t = wp.tile([128, 4, QR, 256], dt.float8e4, tag="w1sw0",
                    name="w1sw0")
        nc.sync.dma_start(t[:], w1swv[:, 0:4, :, :])
        w1swcb.append(t)
        # remaining w1 fp16 + residual stationary per 512-col block,
        # paced to group-0 use
        w1cb = [None]
        w1bb = [w1b0]
        for b in range(1, M1 // 4):
            t = wp.tile([KP, K1A, W], dt.float16, tag=f"w1c{b}",
                        name=f"w1c{b}")
            nc.sync.dma_start(t[:], w1av[:, :, b * W:(b + 1) * W])
            w1cb.append(t)
            t = wp.tile([128, W], dt.float16, tag=f"w1b{b}",
                        name=f"w1b{b}")
            nc.sync.dma_start(t[:], w1b[:, b * W:(b + 1) * W])
            w1bb.append(t)
            t = wp.tile([128, 4, QR, 256], dt.float8e4, tag=f"w1sw{b}",
                        name=f"w1sw{b}")
            nc.sync.dma_start(t[:], w1swv[:, 4 * b:4 * b + 4, :, :])
            w1swcb.append(t)

        def w1v(k, mt):
            b, i = mt // 4, mt % 4
            cs = slice(i * 128, (i + 1) * 128)
            if k == K1A:
                return w1bb[b][:, cs]
            if b == 0:
                return w1k0a[:, cs] if k == 0 else w1ka[:, k - 1, cs]
            return w1cb[b][:, k, cs]

        # x for groups 1..3, one transfer each stream
        xab = wp.tile([KP, K1A, (ng - 1) * W], dt.float16, tag="xab")
        nc.sync.dma_start(xab[:], xa6v[:, :, W:bc])
        xa7b = wp.tile([128, (ng - 1) * W], dt.float16, tag="xa7b")
        nc.sync.dma_start(xa7b[:], xa7[:, W:bc])
        xr123 = wp.tile([128, 2 * QR, (ng - 1) * W], dt.float8e4,
                        tag="xr123")
        nc.sync.dma_start(xr123[:], xrv[:, :, W:bc])

        w2swb = wp.tile([128, K2P, M2, 256], dt.float8e4, tag="w2sw")
        nc.sync.dma_start(w2swb[:], w2swv[:])
        w3swb = wp.tile([128, K3P, M3, 256], dt.float8e4, tag="w3sw")
        nc.sync.dma_start(w3swb[:], w3swv[:])
        w4sb = wp.tile([128, K4T, NCLS], dt.bfloat16, tag="w4")
        nc.sync.dma_start(w4sb[:],
                          w4t.ap().rearrange("(kt p) n -> p kt n", p=128))

        # const views into the packed per-output-tile scale/bias table
        def a1v(mt): return cstb[:, mt:mt + 1]
        def c1v(mt): return cstb[:, M1 + mt:M1 + mt + 1]
        def a2v(mt): return cstb[:, 2 * M1 + mt:2 * M1 + mt + 1]
        def c2v(mt): return cstb[:, 2 * M1 + M2 + mt:2 * M1 + M2 + mt + 1]
        def a3v(mt):
            o = 2 * M1 + 2 * M2
            return cstb[:, o + mt:o + mt + 1]
        def c3v(mt):
            o = 2 * M1 + 2 * M2 + M3
            return cstb[:, o + mt:o + mt + 1]
        b4v = cstb[:, NCST - NCLS:NCST]

        zout = wp.tile([128, ng * 4, NCLS], dt.float32, tag="zout")
        ssum = wp.tile([128, ng * 4], dt.float32, tag="ssum")
        lsum = wp.tile([128, ng * 4], dt.float32, tag="lsum")

        AX = mybir.AxisListType

        def emit_epilogue(lo, hi, dma=True, keep_exp=True):
            # log_softmax over the free dim; |z| is small so no max-shift.
            # Row-sums of exp go through a DVE reduce (a Scalar accum_out
            # costs a serial 278ns ACTIVATION_READ_ACCUMULATOR per tile).
            for r in range(lo, hi):
                e = ep.tile([128, NCLS], dt.float32, tag="e")
                nc.scalar.activation(e[:], zout[:, r, :], AF.Exp)
                nc.vector.tensor_reduce(ssum[:, r:r + 1], e[:],
                                        axis=AX.X, op=ALU.add)
            nc.scalar.activation(lsum[:, lo:hi], ssum[:, lo:hi], AF.Ln)
            if keep_exp:
                # tiny dummy Exp re-loads the Exp activation table (1.28us
                # per table swap) OFF the critical path, so the final
                # group's epilogue starts with the right table resident
                e = ep.tile([128, 1], dt.float32, tag="dum")
                nc.scalar.activation(e[:], lsum[:, lo:lo + 1], AF.Exp)
            for r in range(lo, hi):
                nc.vector.tensor_scalar(zout[:, r, :], zout[:, r, :],
                                        lsum[:, r:r + 1], None,
                                        op0=ALU.subtract)
            if dma:
                nc.sync.dma_start(
                    out.ap()[lo * 128:hi * 128, :].rearrange(
                        "(g p) n -> p g n", p=128),
                    zout[:, lo:hi, :])

        def xav(k, g):
            if g == 0:
                if k == K1A:
                    return x70[:]
                return x00[:] if k == 0 else x0r[:, k - 1, :]
            if k == K1A:
                return xa7b[:, (g - 1) * W:g * W]
            return xab[:, k, (g - 1) * W:g * W]

        for g in range(ng):
            if g > 0:
                # epilogue for the previous group hides under this
                # group's L1 matmuls
                emit_epilogue(4 * (g - 1), 4 * g)

            xrg = xr0 if g == 0 else xr123
            off = 0 if g == 0 else (g - 1) * W

            # ---- L1: [784 -> 3072], 6x112-row + 1x128-row fp16 passes +
            # 3 DRSW fp8 residual passes, all N=512
            h1t = [ap_.tile([128, K2P, W], dt.float8e4, tag="h1a",
                            name="h1a"),
                   ap_.tile([128, K2P, W], dt.float8e4, tag="h1b",
                            name="h1b")]
            for mt in range(M1):
                pt = ps.tile([128, W], dt.float32, tag="ps")
                for k in range(K1A + 1):
                    nc.tensor.matmul(pt[:], w1v(k, mt), xav(k, g),
                                     start=(k == 0), stop=False)
                    if g == 0 and mt == 0 and k == 0:
                        # second warm burst: keeps the PE busy through the
                        # data-wait hole while the rest of the group-0
                        # transfers land
                        for wi in range(NWARM2):
                            nc.tensor.matmul(pwt[:], wrmb[:, 0:128],
                                             wrmb[:], start=(wi == 0),
                                             stop=(wi == NWARM2 - 1))
                for q in range(QR):
                    nc.tensor.matmul(pt[:], w1swcb[mt // 4][:, mt % 4, q, :],
                                     xrg[:, 2 * q:2 * q + 2,
                                         off:off + W],
                                     start=False, stop=(q == QR - 1),
                                     perf_mode=DRSW)
                nc.scalar.activation(h1t[mt // 12][:, mt % 12, :], pt[:],
                                     AF.Sign, bias=c1v(mt), scale=a1v(mt))

            # ---- L2: [3072 -> 1536], fp8 DRSW, N=512
            h2t = [ap_.tile([128, K3P, W], dt.float8e4, tag="h2a",
                            name="h2a"),
                   ap_.tile([128, K3P, W], dt.float8e4, tag="h2b",
                            name="h2b")]
            for mt in range(M2):
                pt = ps.tile([128, W], dt.float32, tag="ps")
                for kp in range(K2P):
                    nc.tensor.matmul(
                        pt[:], w2swb[:, kp, mt, :],
                        h1t[kp // 6][:, 2 * (kp % 6):2 * (kp % 6) + 2, :],
                        start=(kp == 0), stop=(kp == K2P - 1),
                        perf_mode=DRSW)
                nc.scalar.activation(h2t[mt // 6][:, mt % 6, :], pt[:],
                                     AF.Sign, bias=c2v(mt), scale=a2v(mt))

            # ---- L3: [1536 -> 768], fp8 DRSW, N=512; scale/bias on the
            # Scalar engine (Identity), clip on DVE; bf16 output keeps
            # L4 single-pass
            h3t = [ap_.tile([128, K4T // 2, W], dt.bfloat16,
                            tag="h3a", name="h3a"),
                   ap_.tile([128, K4T // 2, W], dt.bfloat16,
                            tag="h3b", name="h3b")]
            for mt in range(M3):
                pt = ps.tile([128, W], dt.float32, tag="ps")
                for kp in range(K3P):
                    nc.tensor.matmul(
                        pt[:], w3swb[:, kp, mt, :],
                        h2t[kp // 3][:, 2 * (kp % 3):2 * (kp % 3) + 2, :],
                        start=(kp == 0), stop=(kp == K3P - 1),
                        perf_mode=DRSW)
                h3v = h3t[mt // 3][:, mt % 3, :]
                nc.scalar.activation(h3v, pt[:], AF.Identity,
                                     bias=c3v(mt), scale=a3v(mt))
                nc.vector.tensor_scalar(h3v, h3v, 1.0, -1.0,
                                        op0=ALU.min, op1=ALU.max)

            # ---- L4: logits z = y3 @ w4.T + b4, [batch-tile, 10]
            for bt in range(4):
                r = 4 * g + bt
                p4 = ps4.tile([128, NCLS], dt.float32, tag="p4")
                for kt in range(K4T):
                    nc.tensor.matmul(p4[:],
                                     h3t[kt // 3][:, kt % 3,
                                         bt * 128:(bt + 1) * 128],
                                     w4sb[:, kt, :],
                                     start=(kt == 0),
                                     stop=(kt == K4T - 1))
                nc.vector.tensor_add(zout[:, r, :], p4[:], b4v)

        # last group's epilogue: batched (one Exp-table residency, one Ln
        # table load) -- per-tile interleaving can't hide behind L4's
        # ~0.6us of matmuls and paid 8x 1.28us table swaps
        emit_epilogue(ng * 4 - 4, ng * 4, dma=True, keep_exp=False)

    nc.finalize()
    return nc


def _interleave(A, Bm):
    """SW-interleave layout for DoubleRowSwInterleave stationary:
    L[:, 0::2] = A[:, ::-1]; L[:, 1::2] = B[:, ::-1]."""
    p, m = A.shape
    L = np.empty((p, 2 * m), A.dtype)
    L[:, 0::2] = A[:, ::-1]
    L[:, 1::2] = Bm[:, ::-1]
    return L


def _sw_image(wt, kpairs, mtiles):
    """Build the [128, kpairs*mtiles*256] SW-interleaved image from
    wt [K, M] (K = 256*kpairs contraction rows, M = 128*mtiles outs)."""
    K, M = wt.shape
    assert K == 256 * kpairs and M == 128 * mtiles
    img = np.empty((128, kpairs * mtiles * 256), np.float32)
    v = img.reshape(128, kpairs, mtiles, 256)
    for q in range(kpairs):
        A = wt[256 * q:256 * q + 128, :]
        Bm = wt[256 * q + 128:256 * q + 256, :]
        for mt in range(mtiles):
            v[:, q, mt, :] = _interleave(A[:, mt * 128:(mt + 1) * 128],
                                         Bm[:, mt * 128:(mt + 1) * 128])
    return img


def _prep(x, w1, b1, w2, b2, w3, b3, w4, b4,
          g1, be1, m1, v1, g2, be2, m2, v2, g3, be3, m3, v3):
    """Host-side prep: transposes, binarized weight casts, BN folds,
    the fp16 + scaled-fp8 split of x, and SW-interleaved fp8 images."""
    import concourse.mybir as mybir
    f8 = mybir.dt.np(mybir.dt.float8e4)
    bf16 = mybir.dt.np(mybir.dt.bfloat16)

    def fold(g, be, m, v, b):
        a = (g / np.sqrt(v + np.float32(BN_EPS))).astype(np.float32)
        c = (a * (b - m) + be).astype(np.float32)
        return a, c

    a1, c1 = fold(g1, be1, m1, v1, b1)
    a2, c2 = fold(g2, be2, m2, v2, b2)
    a3, c3 = fold(g3, be3, m3, v3, b3)

    def cols(v, mtiles):
        return v.reshape(mtiles, 128).T

    cstm = np.zeros((128, NCST), np.float32)
    o = 0
    for v, m in ((a1, M1), (c1, M1), (a2, M2), (c2, M2), (a3, M3), (c3, M3)):
        cstm[:, o:o + m] = cols(v, m)
        o += m
    cstm[:, o:o + NCLS] = b4.astype(np.float32)[None, :]

    s1t = np.sign(w1).T.astype(np.float32)          # [784, 3072]
    # w1a image: rows 0:672 as [112, 6*3072] (k-major per partition)
    w1a = np.ascontiguousarray(
        s1t[0:672].reshape(K1A, KP, H1).transpose(1, 0, 2)
        .reshape(KP, K1A * H1)).astype(np.float16)
    # w1b: rows 672:784 ++ rows 768:784 (for the fp16 residual tail)
    w1b = np.ascontiguousarray(
        np.concatenate([s1t[672:784], s1t[768:784]], axis=0)
    ).astype(np.float16)
    # w1sw: residual stationary rows 0:768, scaled 2^-RSH; mt-major
    w1sw = np.ascontiguousarray(
        _sw_image(s1t[0:768] * np.float32(2.0 ** -RSH), QR, M1)
        .reshape(128, QR, M1, 256).transpose(0, 2, 1, 3)
        .reshape(128, M1 * QR * 256))

    pre = dict(
        w1a=w1a, w1b=w1b,
        w1sw=w1sw.astype(f8),
        w2sw=_sw_image(np.sign(w2).T.astype(np.float32), K2P, M2).astype(f8),
        w3sw=_sw_image(np.sign(w3).T.astype(np.float32), K3P, M3).astype(f8),
        w4t=np.ascontiguousarray(w4.T).astype(bf16),
        cst=cstm,
        wrm=np.zeros((128, W), np.float16),
    )

    x = x.astype(np.float32)
    xa16 = x.astype(np.float16)
    r = x - xa16.astype(np.float32)
    # xa6 image: x.T rows 0:672 as [112, 6*bc-per-core] built per core later
    xa6t = xa16.T[0:672]                            # [672, B]
    # pass-7 rows: x.T[672:784] ++ fp16(r.T[768:784])
    xa7t = np.concatenate([xa16.T[672:784],
                           r.T[768:784].astype(np.float16)], axis=0)
    rb = (r.T[0:768] * np.float32(2.0 ** RSH)).astype(f8)   # [768, B]
    return pre, xa6t, xa7t, rb


def run(inputs, **spmd_kwargs):
    from concourse.bass_utils import run_bass_kernel_spmd

    if "nc" not in _cached:
        _cached["nc"] = _build(BC)
    nc = _cached["nc"]

    inputs = {k: np.asarray(v) for k, v in inputs.items()}
    pre, xa6t, xa7t, rb = _prep(**inputs)

    in_maps = []
    for core in range(NCORES):
        m = dict(pre)
        cs = slice(core * BC, (core + 1) * BC)
        m["xa6"] = np.ascontiguousarray(
            xa6t[:, cs].reshape(K1A, KP, BC).transpose(1, 0, 2)
            .reshape(KP, K1A * BC))
        m["xa7"] = np.ascontiguousarray(xa7t[:, cs])
        # xr image: [128, 6*bc], j = 2q+e -> rows 256q+128e+p
        m["xr"] = np.ascontiguousarray(
            rb[:, cs].reshape(2 * QR, 128, BC).transpose(1, 0, 2)
            .reshape(128, 2 * QR * BC))
        in_maps.append(m)

    res = run_bass_kernel_spmd(nc, in_maps, list(range(NCORES)), **spmd_kwargs)
    outs = [res.results[i]["out"] for i in range(NCORES)]
    return res, np.concatenate(outs, axis=0).astype(np.float32)


def kernel(**inputs):
    return run(inputs)[1]


# revision 14
# speedup vs baseline: 1.2223x; 1.0023x over previous
"""Trainium2 Bass kernel for a binarized 4-layer MLP (eval mode).

Reference computation (per row of x [B=16384, 784]):
  h1 = x @ sign(w1).T + b1;  s1 = sign(bn1(h1))        (clip doesn't change sign)
  h2 = s1 @ sign(w2).T + b2; s2 = sign(bn2(h2))
  h3 = s2 @ sign(w3).T + b3; y3 = clip(bn3(h3), -1, 1)
  z  = y3 @ w4.T + b4;       out = log_softmax(z)

Sharding: pure data-parallel over the batch across 8 NeuronCores
(weights replicated, no collectives).

Numerics:
  - L1 splits x on the HOST into a fp16 main stream plus a scaled fp8
    residual rb = fp8e4((x - fp16(x)) * 2^9); the residual's stationary
    operand is sign(w1) * 2^-9 (exact in fp8e4), accumulated into the
    same fp32 PSUM.  Rows 0:768 of the residual ride 3 DoubleRow
    passes; rows 768:784 ride IN the fp16 stream as extra fp16 rows
    (fp16 of the fp32 remainder -- err 2^-22|x| there), so the 7th
    fp16 pass is 128 rows: x[672:784] ++ r[768:784].  Total L1 error
    <= 2^-15|x| -> 7.6e-3 end-to-end (tolerance 2e-2).
  - L2/L3: operands are +-1/0 in fp8e4 -> DoubleRow matmuls are
    bit-exact integer sums in fp32 PSUM.
  - BN + bias folding: bn(h + b) = A*h + C applied per-partition by the
    Sign/Identity activations (fp32 internally).

Schedule notes (from NTFF trace analysis):
  - All fp8 matmuls use DoubleRowSwInterleave at N=512: the plain-DR
    +0.1ns/col penalty at 512 wide is gone with SW-interleaved weights
    (HW-probed 215.8ns at N=512, same 0.42ns/col as fp16), and the
    ~135ns DR LDWEIGHTS hides under 216ns matmuls -- the baseline's
    256-wide residual paid ~25us of un-hidden LDWEIGHTS bubbles.
  - Weights for SW-interleave are pre-interleaved on the host as exact
    SBUF images (per column pair j: A[:, ::-1]/B[:, ::-1] interleaved),
    so their DMA is a single contiguous transfer.
  - every dma_start costs ~620ns of serialized Sync-engine time, so
    inputs/weights load as FEW large transfers; only the group-0
    block-0 critical path keeps small fine-grained transfers.
  - a dummy-matmul burst on a zero tile warms the PE HAM clock gate
    (1.2 -> 2.4 GHz) while the startup DMAs are still in flight.
  - h1/h2 activations are split into two tiles each so the next
    layer's first matmul depends only on the first half.
  - the log-softmax epilogue for group g-1 hides under group g; in the
    last group it is interleaved per batch-tile behind L4.
"""

import sys

if "/opt/trn_rl_repo" not in sys.path:
    sys.path.insert(0, "/opt/trn_rl_repo")

import numpy as np

D_IN, H1, H2, H3, NCLS = 784, 3072, 1536, 768, 10
B, NCORES = 16384, 8
BC = B // NCORES          # batch rows per core
W = 512                   # batch columns per group
NG = BC // W              # groups per core
KP = 112                  # L1 fp16 k-tile partition size for passes 0-5
K1A = 6                   # fp16 112-row passes (rows 0:672)
QR = 3                    # residual DoubleRow passes (rows 0:768)
M1, M2, M3 = H1 // 128, H2 // 128, H3 // 128   # 24, 12, 6
K2P, K3P = H1 // 256, H2 // 256                # DR pair-pass iters: 12, 6
K4T = H3 // 128                                # 6
NCST = 2 * M1 + 2 * M2 + 2 * M3 + NCLS         # packed per-tile consts: 94
BN_EPS = 1e-5
RSH = 9                   # residual scale: rb = fp8(r * 2^RSH), w = +-2^-RSH
NWARM = 3
NWARM2 = 10

_cached = {}


def _build(bc):
    import concourse.bacc as bacc
    import concourse.mybir as mybir
    import concourse.tile as tile

    dt = mybir.dt
    AF = mybir.ActivationFunctionType
    PM = mybir.MatmulPerfMode
    ALU = mybir.AluOpType
    DRSW = PM.DoubleRowSwInterleave

    ng = bc // W
    nc = bacc.Bacc("TRN2", target_bir_lowering=False, debug=False,
                   num_devices=NCORES)

    # xa6 = fp16(x).T rows 0:672 per core, host image [112, 6*bc]
    xa6 = nc.declare_dram_parameter("xa6", [KP, K1A * bc], dt.float16,
                                    isOutput=False)
    # xa7 = fp16 pass-7 rows: x.T[672:784] ++ r.T[768:784], [128, bc]
    xa7 = nc.declare_dram_parameter("xa7", [128, bc], dt.float16,
                                    isOutput=False)
    # xr = fp8((x - fp16(x)).T[0:768] * 2^RSH) DR-paired, image [128, 6*bc]
    xr = nc.declare_dram_parameter("xr", [128, 2 * QR * bc], dt.float8e4,
                                   isOutput=False)
    # w1a = sign(w1).T rows 0:672 fp16, image [112, 6*3072]
    w1a = nc.declare_dram_parameter("w1a", [KP, K1A * H1], dt.float16,
                                    isOutput=False)
    # w1b = sign(w1).T rows 672:784 ++ 768:784, [128, 3072]
    w1b = nc.declare_dram_parameter("w1b", [128, H1], dt.float16,
                                    isOutput=False)
    # w1sw = residual stationary, SW-interleaved image [128, 24*3*256] fp8
    # (mt-major so per-4-tile-block transfers are contiguous)
    w1sw = nc.declare_dram_parameter("w1sw", [128, M1 * QR * 256],
                                     dt.float8e4, isOutput=False)
    w2sw = nc.declare_dram_parameter("w2sw", [128, K2P * M2 * 256],
                                     dt.float8e4, isOutput=False)
    w3sw = nc.declare_dram_parameter("w3sw", [128, K3P * M3 * 256],
                                     dt.float8e4, isOutput=False)
    w4t = nc.declare_dram_parameter("w4t", [H3, NCLS], dt.bfloat16,
                                    isOutput=False)
    cst = nc.declare_dram_parameter("cst", [128, NCST], dt.float32,
                                    isOutput=False)
    out = nc.declare_dram_parameter("out", [bc, NCLS], dt.float32,
                                    isOutput=True)

    with tile.TileContext(nc) as tc, \
            tc.tile_pool(name="wts", bufs=1) as wp, \
            tc.tile_pool(name="act", bufs=1) as ap_, \
            tc.tile_pool(name="eps", bufs=2) as ep, \
            tc.tile_pool(name="ps", bufs=4, space="PSUM") as ps, \
            tc.tile_pool(name="ps4", bufs=2, space="PSUM") as ps4, \
            tc.tile_pool(name="psw", bufs=1, space="PSUM") as pw:

        # ---- startup-critical transfers first: group-0 x and the first
        # 512-col block of w1, fine-grained; then per-block w1; then bulk.
        xa6v = xa6.ap().rearrange("p (k w) -> p k w", k=K1A)
        w1av = w1a.ap().rearrange("p (k n) -> p k n", k=K1A)
        xrv = xr.ap().rearrange("p (j w) -> p j w", j=2 * QR)
        w1swv = w1sw.ap().rearrange("p (m q n) -> p m q n", m=M1, q=QR)
        w2swv = w2sw.ap().rearrange("p (q m n) -> p q m n", q=K2P, m=M2)
        w3swv = w3sw.ap().rearrange("p (q m n) -> p q m n", q=K3P, m=M3)

        x00 = wp.tile([KP, W], dt.float16, tag="x00")
        nc.sync.dma_start(x00[:], xa6v[:, 0, 0:W])
        w1k0a = wp.tile([KP, W], dt.float16, tag="w1k0a")
        nc.sync.dma_start(w1k0a[:], w1av[:, 0, 0:W])

        # ---- HAM warm-up on x00 itself (no extra DMA/trigger): junk
        # accumulates into a scratch PSUM bank that is never read.
        pwt = pw.tile([128, W], dt.float32, tag="pw")
        for wi in range(NWARM):
            nc.tensor.matmul(pwt[:], x00[:, 0:128], x00[:],
                             start=(wi == 0), stop=(wi == NWARM - 1))
        x0r = wp.tile([KP, K1A - 1, W], dt.float16, tag="x0r")
        nc.sync.dma_start(x0r[:], xa6v[:, 1:K1A, 0:W])
        w1ka = wp.tile([KP, K1A - 1, W], dt.float16, tag="w1ka")
        nc.sync.dma_start(w1ka[:], w1av[:, 1:K1A, 0:W])
        x70 = wp.tile([128, W], dt.float16, tag="x70")
        nc.sync.dma_start(x70[:], xa7[:, 0:W])
        w1b0 = wp.tile([128, W], dt.float16, tag="w1b0")
        nc.sync.dma_start(w1b0[:], w1b[:, 0:W])
        cstb = wp.tile([128, NCST], dt.float32, tag="cst")
        nc.sync.dma_start(cstb[:], cst[:])
        xr0 = wp.tile([128, 2 * QR, W], dt.float8e4, tag="xr0")
        nc.sync.dma_start(xr0[:], xrv[:, :, 0:W])
        # residual stationary per 4-tile block (block 0 first)
        w1swcb = []
        t = wp.tile([128, 4, QR, 256], dt.float8e4, tag="w1sw0",
                    name="w1sw0")
        nc.sync.dma_start(t[:], w1swv[:, 0:4, :, :])
        w1swcb.append(t)
        # remaining w1 fp16 + residual stationary per 512-col block,
        # paced to group-0 use
        w1cb = [None]
        w1bb = [w1b0]
        for b in range(1, M1 // 4):
            t = wp.tile([KP, K1A, W], dt.float16, tag=f"w1c{b}",
                        name=f"w1c{b}")
            nc.sync.dma_start(t[:], w1av[:, :, b * W:(b + 1) * W])
            w1cb.append(t)
            t = wp.tile([128, W], dt.float16, tag=f"w1b{b}",
                        name=f"w1b{b}")
            nc.sync.dma_start(t[:], w1b[:, b * W:(b + 1) * W])
            w1bb.append(t)
            t = wp.tile([128, 4, QR, 256], dt.float8e4, tag=f"w1sw{b}",
                        name=f"w1sw{b}")
            nc.sync.dma_start(t[:], w1swv[:, 4 * b:4 * b + 4, :, :])
            w1swcb.append(t)

        def w1v(k, mt):
            b, i = mt // 4, mt % 4
            cs = slice(i * 128, (i + 1) * 128)
            if k == K1A:
                return w1bb[b][:, cs]
            if b == 0:
                return w1k0a[:, cs] if k == 0 else w1ka[:, k - 1, cs]
            return w1cb[b][:, k, cs]

        # x for groups 1..3, one transfer each stream
        xab = wp.tile([KP, K1A, (ng - 1) * W], dt.float16, tag="xab")
        nc.sync.dma_start(xab[:], xa6v[:, :, W:bc])
        xa7b = wp.tile([128, (ng - 1) * W], dt.float16, tag="xa7b")
        nc.sync.dma_start(xa7b[:], xa7[:, W:bc])
        xr123 = wp.tile([128, 2 * QR, (ng - 1) * W], dt.float8e4,
                        tag="xr123")
        nc.sync.dma_start(xr123[:], xrv[:, :, W:bc])

        w2swb = wp.tile([128, K2P, M2, 256], dt.float8e4, tag="w2sw")
        nc.sync.dma_start(w2swb[:], w2swv[:])
        w3swb = wp.tile([128, K3P, M3, 256], dt.float8e4, tag="w3sw")
        nc.sync.dma_start(w3swb[:], w3swv[:])
        w4sb = wp.tile([128, K4T, NCLS], dt.bfloat16, tag="w4")
        nc.sync.dma_start(w4sb[:],
                          w4t.ap().rearrange("(kt p) n -> p kt n", p=128))

        # const views into the packed per-output-tile scale/bias table
        def a1v(mt): return cstb[:, mt:mt + 1]
        def c1v(mt): return cstb[:, M1 + mt:M1 + mt + 1]
        def a2v(mt): return cstb[:, 2 * M1 + mt:2 * M1 + mt + 1]
        def c2v(mt): return cstb[:, 2 * M1 + M2 + mt:2 * M1 + M2 + mt + 1]
        def a3v(mt):
            o = 2 * M1 + 2 * M2
            return cstb[:, o + mt:o + mt + 1]
        def c3v(mt):
            o = 2 * M1 + 2 * M2 + M3
            return cstb[:, o + mt:o + mt + 1]
        b4v = cstb[:, NCST - NCLS:NCST]

        zout = wp.tile([128, ng * 4, NCLS], dt.float32, tag="zout")
        ssum = wp.tile([128, ng * 4], dt.float32, tag="ssum")
        lsum = wp.tile([128, ng * 4], dt.float32, tag="lsum")

        AX = mybir.AxisListType

        def emit_epilogue(lo, hi, dma=True, keep_exp=True):
            # log_softmax over the free dim; |z| is small so no max-shift.
            # Row-sums of exp go through a DVE reduce (a Scalar accum_out
            # costs a serial 278ns ACTIVATION_READ_ACCUMULATOR per tile).
            for r in range(lo, hi):
                e = ep.tile([128, NCLS], dt.float32, tag="e")
                nc.scalar.activation(e[:], zout[:, r, :], AF.Exp)
                nc.vector.tensor_reduce(ssum[:, r:r + 1], e[:],
                                        axis=AX.X, op=ALU.add)
            nc.scalar.activation(lsum[:, lo:hi], ssum[:, lo:hi], AF.Ln)
            if keep_exp:
                # tiny dummy Exp re-loads the Exp activation table (1.28us
                # per table swap) OFF the critical path, so the final
                # group's epilogue starts with the right table resident
                e = ep.tile([128, 1], dt.float32, tag="dum")
                nc.scalar.activation(e[:], lsum[:, lo:lo + 1], AF.Exp)
            for r in range(lo, hi):
                nc.vector.tensor_scalar(zout[:, r, :], zout[:, r, :],
                                        lsum[:, r:r + 1], None,
                                        op0=ALU.subtract)
            if dma:
                nc.sync.dma_start(
                    out.ap()[lo * 128:hi * 128, :].rearrange(
                        "(g p) n -> p g n", p=128),
                    zout[:, lo:hi, :])

        def xav(k, g):
            if g == 0:
                if k == K1A:
                    return x70[:]
                return x00[:] if k == 0 else x0r[:, k - 1, :]
            if k == K1A:
                return xa7b[:, (g - 1) * W:g * W]
            return xab[:, k, (g - 1) * W:g * W]

        for g in range(ng):
            if g > 0:
                # epilogue for the previous group hides under this
                # group's L1 matmuls
                emit_epilogue(4 * (g - 1), 4 * g)

            xrg = xr0 if g == 0 else xr123
            off = 0 if g == 0 else (g - 1) * W

            # ---- L1: [784 -> 3072], 6x112-row + 1x128-row fp16 passes +
            # 3 DRSW fp8 residual passes, all N=512
            h1t = [ap_.tile([128, K2P, W], dt.float8e4, tag="h1a",
                            name="h1a"),
                   ap_.tile([128, K2P, W], dt.float8e4, tag="h1b",
                            name="h1b")]
            for mt in range(M1):
                pt = ps.tile([128, W], dt.float32, tag="ps")
                for k in range(K1A + 1):
                    nc.tensor.matmul(pt[:], w1v(k, mt), xav(k, g),
                                     start=(k == 0), stop=False)
                    if g == 0 and mt == 0 and k == 0:
                        # second warm burst: keeps the PE busy through the
                        # data-wait hole while the rest of the group-0
                        # transfers land
                        for wi in range(NWARM2):
                            nc.tensor.matmul(pwt[:], x00[:, 0:128],
                                             x00[:], start=(wi == 0),
                                             stop=(wi == NWARM2 - 1))
                for q in range(QR):
                    nc.tensor.matmul(pt[:], w1swcb[mt // 4][:, mt % 4, q, :],
                                     xrg[:, 2 * q:2 * q + 2,
                                         off:off + W],
                                     start=False, stop=(q == QR - 1),
                                     perf_mode=DRSW)
                nc.scalar.activation(h1t[mt // 12][:, mt % 12, :], pt[:],
                                     AF.Sign, bias=c1v(mt), scale=a1v(mt))

            # ---- L2: [3072 -> 1536], fp8 DRSW, N=512
            h2t = [ap_.tile([128, K3P, W], dt.float8e4, tag="h2a",
                            name="h2a"),
                   ap_.tile([128, K3P, W], dt.float8e4, tag="h2b",
                            name="h2b")]
            for mt in range(M2):
                pt = ps.tile([128, W], dt.float32, tag="ps")
                for kp in range(K2P):
                    nc.tensor.matmul(
                        pt[:], w2swb[:, kp, mt, :],
                        h1t[kp // 6][:, 2 * (kp % 6):2 * (kp % 6) + 2, :],
                        start=(kp == 0), stop=(kp == K2P - 1),
                        perf_mode=DRSW)
                nc.scalar.activation(h2t[mt // 6][:, mt % 6, :], pt[:],
                                     AF.Sign, bias=c2v(mt), scale=a2v(mt))

            # ---- L3: [1536 -> 768], fp8 DRSW, N=512; scale/bias on the
            # Scalar engine (Identity), clip on DVE; bf16 output keeps
            # L4 single-pass
            h3t = [ap_.tile([128, K4T // 2, W], dt.bfloat16,
                            tag="h3a", name="h3a"),
                   ap_.tile([128, K4T // 2, W], dt.bfloat16,
                            tag="h3b", name="h3b")]
            for mt in range(M3):
                pt = ps.tile([128, W], dt.float32, tag="ps")
                for kp in range(K3P):
                    nc.tensor.matmul(
                        pt[:], w3swb[:, kp, mt, :],
                        h2t[kp // 3][:, 2 * (kp % 3):2 * (kp % 3) + 2, :],
                        start=(kp == 0), stop=(kp == K3P - 1),
                        perf_mode=DRSW)
                h3v = h3t[mt // 3][:, mt % 3, :]
                nc.scalar.activation(h3v, pt[:], AF.Identity,
                                     bias=c3v(mt), scale=a3v(mt))
                nc.vector.tensor_scalar(h3v, h3v, 1.0, -1.0,
                                        op0=ALU.min, op1=ALU.max)

            # ---- L4: logits z = y3 @ w4.T + b4, [batch-tile, 10]
            for bt in range(4):
                r = 4 * g + bt
                p4 = ps4.tile([128, NCLS], dt.float32, tag="p4")
                for kt in range(K4T):
                    nc.tensor.matmul(p4[:],
                                     h3t[kt // 3][:, kt % 3,
                                         bt * 128:(bt + 1) * 128],
                                     w4sb[:, kt, :],
                                     start=(kt == 0),
                                     stop=(kt == K4T - 1))
                nc.vector.tensor_add(zout[:, r, :], p4[:], b4v)

        # last group's epilogue: batched (one Exp-table residency, one Ln
        # table load) -- per-tile interleaving can't hide behind L4's
        # ~0.6us of matmuls and paid 8x 1.28us table swaps
        emit_epilogue(ng * 4 - 4, ng * 4, dma=True, keep_exp=False)

    nc.finalize()
    return nc


def _interleave(A, Bm):
    """SW-interleave layout for DoubleRowSwInterleave stationary:
    L[:, 0::2] = A[:, ::-1]; L[:, 1::2] = B[:, ::-1]."""
    p, m = A.shape
    L = np.empty((p, 2 * m), A.dtype)
    L[:, 0::2] = A[:, ::-1]
    L[:, 1::2] = Bm[:, ::-1]
    return L


def _sw_image(wt, kpairs, mtiles):
    """Build the [128, kpairs*mtiles*256] SW-interleaved image from
    wt [K, M] (K = 256*kpairs contraction rows, M = 128*mtiles outs)."""
    K, M = wt.shape
    assert K == 256 * kpairs and M == 128 * mtiles
    img = np.empty((128, kpairs * mtiles * 256), np.float32)
    v = img.reshape(128, kpairs, mtiles, 256)
    for q in range(kpairs):
        A = wt[256 * q:256 * q + 128, :]
        Bm = wt[256 * q + 128:256 * q + 256, :]
        for mt in range(mtiles):
            v[:, q, mt, :] = _interleave(A[:, mt * 128:(mt + 1) * 128],
                                         Bm[:, mt * 128:(mt + 1) * 128])
    return img


def _prep(x, w1, b1, w2, b2, w3, b3, w4, b4,
          g1, be1, m1, v1, g2, be2, m2, v2, g3, be3, m3, v3):
    """Host-side prep: transposes, binarized weight casts, BN folds,
    the fp16 + scaled-fp8 split of x, and SW-interleaved fp8 images."""
    import concourse.mybir as mybir
    f8 = mybir.dt.np(mybir.dt.float8e4)
    bf16 = mybir.dt.np(mybir.dt.bfloat16)

    def fold(g, be, m, v, b):
        a = (g / np.sqrt(v + np.float32(BN_EPS))).astype(np.float32)
        c = (a * (b - m) + be).astype(np.float32)
        return a, c

    a1, c1 = fold(g1, be1, m1, v1, b1)
    a2, c2 = fold(g2, be2, m2, v2, b2)
    a3, c3 = fold(g3, be3, m3, v3, b3)

    def cols(v, mtiles):
        return v.reshape(mtiles, 128).T

    cstm = np.zeros((128, NCST), np.float32)
    o = 0
    for v, m in ((a1, M1), (c1, M1), (a2, M2), (c2, M2), (a3, M3), (c3, M3)):
        cstm[:, o:o + m] = cols(v, m)
        o += m
    cstm[:, o:o + NCLS] = b4.astype(np.float32)[None, :]

    s1t = np.sign(w1).T.astype(np.float32)          # [784, 3072]
    # w1a image: rows 0:672 as [112, 6*3072] (k-major per partition)
    w1a = np.ascontiguousarray(
        s1t[0:672].reshape(K1A, KP, H1).transpose(1, 0, 2)
        .reshape(KP, K1A * H1)).astype(np.float16)
    # w1b: rows 672:784 ++ rows 768:784 (for the fp16 residual tail)
    w1b = np.ascontiguousarray(
        np.concatenate([s1t[672:784], s1t[768:784]], axis=0)
    ).astype(np.float16)
    # w1sw: residual stationary rows 0:768, scaled 2^-RSH; mt-major
    w1sw = np.ascontiguousarray(
        _sw_image(s1t[0:768] * np.float32(2.0 ** -RSH), QR, M1)
        .reshape(128, QR, M1, 256).transpose(0, 2, 1, 3)
        .reshape(128, M1 * QR * 256))

    pre = dict(
        w1a=w1a, w1b=w1b,
        w1sw=w1sw.astype(f8),
        w2sw=_sw_image(np.sign(w2).T.astype(np.float32), K2P, M2).astype(f8),
        w3sw=_sw_image(np.sign(w3).T.astype(np.float32), K3P, M3).astype(f8),
        w4t=np.ascontiguousarray(w4.T).astype(bf16),
        cst=cstm,
    )

    x = x.astype(np.float32)
    xa16 = x.astype(np.float16)
    r = x - xa16.astype(np.float32)
    # xa6 image: x.T rows 0:672 as [112, 6*bc-per-core] built per core later
    xa6t = xa16.T[0:672]                            # [672, B]
    # pass-7 rows: x.T[672:784] ++ fp16(r.T[768:784])
    xa7t = np.concatenate([xa16.T[672:784],
                           r.T[768:784].astype(np.float16)], axis=0)
    rb = (r.T[0:768] * np.float32(2.0 ** RSH)).astype(f8)   # [768, B]
    return pre, xa6t, xa7t, rb


def run(inputs, **spmd_kwargs):
    from concourse.bass_utils import run_bass_kernel_spmd

    if "nc" not in _cached:
        _cached["nc"] = _build(BC)
    nc = _cached["nc"]

    inputs = {k: np.asarray(v) for k, v in inputs.items()}
    pre, xa6t, xa7t, rb = _prep(**inputs)

    in_maps = []
    for core in range(NCORES):
        m = dict(pre)
        cs = slice(core * BC, (core + 1) * BC)
        m["xa6"] = np.ascontiguousarray(
            xa6t[:, cs].reshape(K1A, KP, BC).transpose(1, 0, 2)
            .reshape(KP, K1A * BC))
        m["xa7"] = np.ascontiguousarray(xa7t[:, cs])
        # xr image: [128, 6*bc], j = 2q+e -> rows 256q+128e+p
        m["xr"] = np.ascontiguousarray(
            rb[:, cs].reshape(2 * QR, 128, BC).transpose(1, 0, 2)
            .reshape(128, 2 * QR * BC))
        in_maps.append(m)

    res = run_bass_kernel_spmd(nc, in_maps, list(range(NCORES)), **spmd_kwargs)
    outs = [res.results[i]["out"] for i in range(NCORES)]
    return res, np.concatenate(outs, axis=0).astype(np.float32)


def kernel(**inputs):
    return run(inputs)[1]
